# revision 1
# baseline (speedup 1.0000x reference)
"""Self-contained Trainium2 attention-block kernel (8 NeuronCores, SPMD).

Problem: x[4,4096,128], Wq/Wk[64,128], Wv[128,128] ->
  softmax((x Wq^T)(x Wk^T)^T / 8) (x Wv^T)   -> [4,4096,128] f32

Sharding: data-parallel over batch (4) x query-halves (2) = 8 cores.
Each core: q rows 2048, full K/V (4096) recomputed locally. No collectives.

Per-core pipeline (all matmuls bf16):
  scores transposed ST[k,q] = KT_chunk.T @ QT (row-tiled 64-contraction
  pairs run concurrently); exp on ScalarE with fused 1/8 scale (no
  max-subtract: scores ~N(0,1)); PV accumulated as outT[v,q] over 32
  k-chunks; softmax denominator via bf16 add-tree (DVE + some GpSimd);
  DMA-transposes flip outT and D partials back to [q,...]; final
  per-partition 1/D scale; bf16 DRAM out upconverted on host.
ST emission runs two groups ahead of PV so the in-order PE never waits
on the exp of the current group.
"""

import sys

sys.path.insert(0, "/opt/trn_rl_repo")

from contextlib import ExitStack

import ml_dtypes
import numpy as np

import concourse.bass as bass  # noqa: F401
import concourse.bacc as bacc
import concourse.tile as tile
from concourse import mybir
from concourse.bass_utils import run_bass_kernel_spmd

BF16 = mybir.dt.bfloat16
F32 = mybir.dt.float32
NPBF16 = ml_dtypes.bfloat16

B, S, D, A = 4, 4096, 128, 64
NQ = S // 2          # q rows per core
QB = 512             # q block (psum bank free size)
KC = 128             # k chunk (matmul contraction tile)
NKC = S // KC        # 32 chunks
NQB = NQ // QB       # 4 q blocks
GROUP = 2            # k chunks per exp group ([128,1024] psum tile)
NGRP = NKC // GROUP  # 16 groups per block
EXP = mybir.ActivationFunctionType.Exp
AXX = mybir.AxisListType.X

_CACHED_NC = None


def _log(msg):
    import time as _t
    print(f"[kernel {_t.strftime('%H:%M:%S')}] {msg}", file=sys.stderr, flush=True)


def build_nc():
    _log("build_nc: tracing graph")
    nc = bacc.Bacc(
        "TRN2", target_bir_lowering=False, debug=False,
        enable_asserts=False, num_devices=8,
    )
    xT = nc.dram_tensor("xT", [D, S], BF16, kind="ExternalInput").ap()
    xqT = nc.dram_tensor("xqT", [D, NQ], BF16, kind="ExternalInput").ap()
    wqTd = nc.dram_tensor("wqTd", [D, 128], BF16, kind="ExternalInput").ap()
    wkTd = nc.dram_tensor("wkTd", [D, 128], BF16, kind="ExternalInput").ap()
    wvT = nc.dram_tensor("wvT", [D, D], BF16, kind="ExternalInput").ap()
    ones = nc.dram_tensor("ones", [D, D], BF16, kind="ExternalInput").ap()
    # outT layout [v, q]; host transposes for free during gather
    out = nc.dram_tensor("out", [D, NQ], BF16, kind="ExternalOutput").ap()

    with tile.TileContext(nc) as tc, ExitStack() as ctx:
        persist = ctx.enter_context(tc.tile_pool(name="persist", bufs=1))
        # PSUM: st 2x(2 banks) + pv 2x(1) + proj 2x(1) = 8 banks
        ps_st = ctx.enter_context(tc.tile_pool(name="ps_st", bufs=2, space="PSUM"))
        ps_pv = ctx.enter_context(tc.tile_pool(name="ps_pv", bufs=2, space="PSUM"))
        ps_pj = ctx.enter_context(tc.tile_pool(name="ps_pj", bufs=2, space="PSUM"))
        ppool = ctx.enter_context(tc.tile_pool(name="ppool", bufs=4))
        tpool = ctx.enter_context(tc.tile_pool(name="tpool", bufs=8))
        mpool = ctx.enter_context(tc.tile_pool(name="mpool", bufs=4))

        # ---- persistent SBUF + input DMAs ----
        xqT_s = persist.tile([D, NQ], BF16, tag="xqT_s")
        nc.sync.dma_start(xqT_s[:, 0:QB], xqT[:, 0:QB])
        xT_s = persist.tile([D, S], BF16, tag="xT_s")
        for j in range(4):  # split so proj matmuls unblock progressively
            nc.sync.dma_start(xT_s[:, j * 1024:(j + 1) * 1024],
                              xT[:, j * 1024:(j + 1) * 1024])
        nc.sync.dma_start(xqT_s[:, QB:], xqT[:, QB:])
        wq_s = persist.tile([D, 128], BF16, tag="wq_s")
        nc.sync.dma_start(wq_s[:], wqTd[:])
        wk_s = persist.tile([D, 128], BF16, tag="wk_s")
        nc.sync.dma_start(wk_s[:], wkTd[:])
        wv_s = persist.tile([D, D], BF16, tag="wv_s")
        nc.sync.dma_start(wv_s[:], wvT[:])
        ones_s = persist.tile([D, D], BF16, tag="ones_s")
        nc.sync.dma_start(ones_s[:], ones[:])

        KT_s = persist.tile([128, S], BF16, tag="KT_s")   # duplicated halves
        QT_s = persist.tile([128, NQ], BF16, tag="QT_s")  # duplicated halves
        V_s = persist.tile([128, S], BF16, tag="V_s")     # [:,c*128:+128]=V[c*128:+128,:]

        # prewarm the exp table so the first real exp doesn't eat the
        # ~2.7us ACT_TABLE_LOAD on the critical path
        warm = persist.tile([1, 1], F32, tag="warm")
        nc.gpsimd.memset(warm[:], 1.0)
        warm2 = persist.tile([1, 1], F32, tag="warm2")
        nc.scalar.activation(warm2[:], warm[:], EXP)

        # ---- projections ----
        # early (ACT copies): QT block0 + KT j0 unblock the first STs fast
        def proj_mm(dst, w, src_slice, copy_engine):
            pt = ps_pj.tile([128, QB], F32, tag="pj")
            nc.tensor.matmul(pt[:], w, src_slice, start=True, stop=True)
            copy_engine(dst, pt[:])

        act_cp = nc.scalar.copy
        dve_cp = nc.vector.tensor_copy
        proj_mm(QT_s[:, 0:QB], wq_s[:], xqT_s[:, 0:QB], act_cp)
        proj_mm(KT_s[:, 0:QB], wk_s[:], xT_s[:, 0:QB], act_cp)
        proj_mm(KT_s[:, QB:2 * QB], wk_s[:], xT_s[:, QB:2 * QB], act_cp)
        # V projection: 4 chunks of [s=128, v=128] per psum tile
        for g in range(8):
            pt = ps_pj.tile([128, QB], F32, tag="pj")
            for i in range(4):
                c = g * 4 + i
                nc.tensor.matmul(pt[:, i * 128:(i + 1) * 128],
                                 xT_s[:, c * 128:(c + 1) * 128], wv_s[:],
                                 start=True, stop=True)
            dve_cp(V_s[:, g * QB:(g + 1) * QB], pt[:])
        for j in range(2, S // QB):
            proj_mm(KT_s[:, j * QB:(j + 1) * QB], wk_s[:],
                    xT_s[:, j * QB:(j + 1) * QB], dve_cp)
        for j in range(1, NQ // QB):
            proj_mm(QT_s[:, j * QB:(j + 1) * QB], wq_s[:],
                    xqT_s[:, j * QB:(j + 1) * QB], dve_cp)

        # ---- attention: flat software pipeline over (qblock, group) ----
        ALL = [(qb, g) for qb in range(NQB) for g in range(NGRP)]

        def emit_st(qb, g):
            q0 = qb * QB
            st = ps_st.tile([128, GROUP * QB], F32, tag="st")
            for i in range(GROUP):
                kc = g * GROUP + i
                h = kc % 2  # row-tile half: concurrent 64-contraction pairs
                lhsT = KT_s[h * 64:(h + 1) * 64, kc * KC:(kc + 1) * KC]
                rhs = QT_s[h * 64:(h + 1) * 64, q0:q0 + QB]
                nc.tensor.matmul(st[:, i * QB:(i + 1) * QB], lhsT, rhs,
                                 start=True, stop=True)
            return st

        st_tiles = {}
        st_tiles[ALL[0]] = emit_st(*ALL[0])
        st_tiles[ALL[1]] = emit_st(*ALL[1])

        pv_tiles = {}
        stacks = {qb: [] for qb in range(NQB)}  # binary-counter D trees
        nadd = {qb: 0 for qb in range(NQB)}

        def tree_add(qb, dst, a, b_):
            # route a fraction of adds to the otherwise-idle GpSimd
            eng = nc.gpsimd if nadd[qb] % 4 == 3 else nc.vector
            eng.tensor_add(dst, a, b_)
            nadd[qb] += 1

        def push(qb, t, lvl):
            st_ = stacks[qb]
            while st_ and st_[-1][0] == lvl:
                plvl, pt_ = st_.pop()
                nt = tpool.tile([128, QB], BF16, tag="tr")
                tree_add(qb, nt[:], pt_[:], t[:])
                t, lvl = nt, plvl + 1
            st_.append((lvl, t))

        def finish_block(qb):
            q0 = qb * QB
            # collapse D tree
            st_ = stacks[qb]
            while len(st_) > 1:
                l0, t0 = st_.pop()
                l1, t_1 = st_.pop()
                nt = tpool.tile([128, QB], BF16, tag="tr")
                tree_add(qb, nt[:], t_1[:], t0[:])
                st_.append((max(l0, l1) + 1, nt))
            s_tile = st_[0][1]  # [k-lane, q] bf16 chunk-sum

            # D[q] via ones-matmul: every output row = sum over k-lanes,
            # so the result arrives already partition-broadcast.
            dps = ps_pj.tile([128, QB], F32, tag="pj", name=f"dps{qb}")
            nc.tensor.matmul(dps[:], ones_s[:], s_tile[:], start=True, stop=True)
            dinvb = mpool.tile([128, QB], F32, tag="dinvb")
            nc.vector.reciprocal_approx_fast(dinvb[:], dps[:])
            outf = mpool.tile([128, QB], BF16, tag="outf")
            nc.vector.tensor_mul(outf[:], pv_tiles[qb][:], dinvb[:])
            nc.sync.dma_start(out[:, q0:q0 + QB], outf[:])
            del pv_tiles[qb]

        FINISH_DELAY = 3  # groups of the next block emitted before a finish
        for idx, (qb, g) in enumerate(ALL):
            st = st_tiles.pop((qb, g))
            p = ppool.tile([128, GROUP * QB], BF16, tag="p")
            nc.scalar.activation(p[:], st[:], EXP, scale=0.125)
            if qb not in pv_tiles:
                pv_tiles[qb] = ps_pv.tile([128, QB], F32, tag="pv",
                                          name=f"pv{qb}")
            pv = pv_tiles[qb]
            for i in range(GROUP):
                kc = g * GROUP + i
                nc.tensor.matmul(pv[:], V_s[:, kc * KC:(kc + 1) * KC],
                                 p[:, i * QB:(i + 1) * QB],
                                 start=(kc == 0), stop=(kc == NKC - 1))
            if idx + 2 < len(ALL):
                st_tiles[ALL[idx + 2]] = emit_st(*ALL[idx + 2])
            t1 = tpool.tile([128, QB], BF16, tag="tr")
            tree_add(qb, t1[:], p[:, 0:QB], p[:, QB:2 * QB])
            push(qb, t1, 1)
            if g == FINISH_DELAY - 1 and qb > 0:
                finish_block(qb - 1)
        finish_block(NQB - 1)

    _log("build_nc: bacc compile")
    nc.compile()
    _log("build_nc: done")
    return nc


def _host_prep(x, Wq, Wk, Wv):
    x = np.asarray(x, dtype=np.float32)
    Wq = np.asarray(Wq, dtype=np.float32)
    Wk = np.asarray(Wk, dtype=np.float32)
    Wv = np.asarray(Wv, dtype=np.float32)
    wqTd = np.ascontiguousarray(
        np.concatenate([Wq.T, Wq.T], axis=1)).astype(NPBF16)
    wkTd = np.ascontiguousarray(
        np.concatenate([Wk.T, Wk.T], axis=1)).astype(NPBF16)
    wvT = np.ascontiguousarray(Wv.T).astype(NPBF16)
    ones = np.ones((D, D), dtype=NPBF16)
    in_maps = []
    for c in range(8):
        b, h = c // 2, c % 2
        in_maps.append({
            "xT": np.ascontiguousarray(x[b].T).astype(NPBF16),
            "xqT": np.ascontiguousarray(
                x[b, h * NQ:(h + 1) * NQ].T).astype(NPBF16),
            "wqTd": wqTd, "wkTd": wkTd, "wvT": wvT, "ones": ones,
        })
    return in_maps


def run(x, Wq, Wk, Wv, trace=False, **kw):
    global _CACHED_NC
    if _CACHED_NC is None:
        _CACHED_NC = build_nc()
    in_maps = _host_prep(x, Wq, Wk, Wv)
    _log("run_bass_kernel_spmd (includes NEFF compile on first call)")
    res = run_bass_kernel_spmd(
        _CACHED_NC, in_maps, core_ids=list(range(8)), trace=trace, **kw)
    _log("run_bass_kernel_spmd returned")
    full = np.zeros((B, S, D), np.float32)
    for c in range(8):
        b, h = c // 2, c % 2
        full[b, h * NQ:(h + 1) * NQ] = np.asarray(
            res.results[c]["out"]).astype(np.float32).T
    return full, res


def kernel(x, Wq, Wk, Wv):
    full, _ = run(x, Wq, Wk, Wv, trace=False)
    return full



# revision 8
# speedup vs baseline: 1.2035x; 1.2035x over previous
"""Self-contained Trainium2 attention-block kernel (8 NeuronCores, SPMD).

Problem: x[4,4096,128], Wq/Wk[64,128], Wv[128,128] ->
  softmax((x Wq^T)(x Wk^T)^T / 8) (x Wv^T)   -> [4,4096,128] f32

Sharding: data-parallel over batch (4) x query-halves (2) = 8 cores.
Each core: q rows 2048, full K (4096) via algebra below. No collectives.

Algebraic reformulation (projections folded into attention):
  scores[k,q] = x_k^T (Wk^T Wq) x_q = xT_chunk.T @ QW,
    QW = M^T x_q with M^T = Wq^T Wk precomputed on host  -> no K proj.
  out[v,q] = Wv^T R / D with R[d,q] = sum_k x[k,d] p[k,q] accumulated
    like PV over k chunks                                -> no V proj.
Both ST and R matmuls contract over the full 128 dim with lhsT taken
straight from DMA'd x layouts (xT [d,k] and x_kd [k, c*128+d]).

Per-core pipeline (all matmuls bf16), ACT-exp-bound steady state:
  ST[k,q] psum ring-3 x [128,1024] (6 banks) + R psum 2x1 bank = 8.
  exp on ScalarE with fused 1/8 scale (no max-subtract: scores ~N(0,1)).
  Softmax denominator: bf16 binary-counter tree on DVE over groups
  0..14; the last group's two chunks fold into the ones-matmul psum
  accumulation (dps), which shares a borrowed ST-ring slot with the
  Wv^T R output. Inputs stream on four DMA queues, weights first.
"""

import sys

sys.path.insert(0, "/opt/trn_rl_repo")

from contextlib import ExitStack

import ml_dtypes
import numpy as np

import concourse.bass as bass  # noqa: F401
import concourse.bacc as bacc
import concourse.tile as tile
from concourse import mybir
from concourse.bass_utils import run_bass_kernel_spmd

BF16 = mybir.dt.bfloat16
F32 = mybir.dt.float32
NPBF16 = ml_dtypes.bfloat16

B, S, D, A = 4, 4096, 128, 64
NQ = S // 2          # q rows per core
QB = 512             # q block (psum bank free size)
KC = 128             # k chunk (matmul contraction tile)
NKC = S // KC        # 32 chunks
NQB = NQ // QB       # 4 q blocks
GROUP = 2            # k chunks per exp group ([128,1024] psum tile)
NGRP = NKC // GROUP  # 16 groups per block
EXP = mybir.ActivationFunctionType.Exp

_CACHED_NC = None


def _log(msg):
    import time as _t
    print(f"[kernel {_t.strftime('%H:%M:%S')}] {msg}", file=sys.stderr, flush=True)


def build_nc():
    _log("build_nc: tracing graph")
    nc = bacc.Bacc(
        "TRN2", target_bir_lowering=False, debug=False,
        enable_asserts=False, num_devices=8,
    )
    xT = nc.dram_tensor("xT", [D, S], BF16, kind="ExternalInput").ap()
    xkd = nc.dram_tensor("xkd", [128, S], BF16, kind="ExternalInput").ap()
    xqT = nc.dram_tensor("xqT", [D, NQ], BF16, kind="ExternalInput").ap()
    # mT = Wq^T Wk | wvT = Wv^T | ones, packed: one small DMA, first
    wpack = nc.dram_tensor("wpack", [D, 384], BF16, kind="ExternalInput").ap()
    # outT layout [v, q]; host transposes for free during gather
    out = nc.dram_tensor("out", [D, NQ], BF16, kind="ExternalOutput").ap()

    with tile.TileContext(nc) as tc, ExitStack() as ctx:
        persist = ctx.enter_context(tc.tile_pool(name="persist", bufs=1))
        # PSUM: st ring 3x(2 banks) + R 2x(1 bank) = 8 banks.
        # dps/WvR and QW-proj tiles borrow ring slots via matching tags.
        ps_st = ctx.enter_context(tc.tile_pool(name="ps_st", bufs=3, space="PSUM"))
        ps_r = ctx.enter_context(tc.tile_pool(name="ps_r", bufs=2, space="PSUM"))
        ppool = ctx.enter_context(tc.tile_pool(name="ppool", bufs=5))
        tpool = ctx.enter_context(tc.tile_pool(name="tpool", bufs=10))
        mpool = ctx.enter_context(tc.tile_pool(name="mpool", bufs=2))

        # ---- persistent SBUF + input DMAs (four queues, weights first) ----
        wp_s = persist.tile([D, 384], BF16, tag="wp_s")
        nc.scalar.dma_start(wp_s[:], wpack[:])
        xqT_s = persist.tile([D, NQ], BF16, tag="xqT_s")
        nc.scalar.dma_start(xqT_s[:, 0:QB], xqT[:, 0:QB])
        nc.scalar.dma_start(xqT_s[:, QB:], xqT[:, QB:])
        # prewarm the exp table before the gpsimd queue fills with DMA
        # enqueues, so the first real exp doesn't eat the ACT_TABLE_LOAD
        warm = persist.tile([1, 1], F32, tag="warm")
        nc.gpsimd.memset(warm[:], 1.0)
        warm2 = persist.tile([1, 1], F32, tag="warm2")
        nc.scalar.activation(warm2[:], warm[:], EXP)

        xT_s = persist.tile([D, S], BF16, tag="xT_s")
        xkd_s = persist.tile([128, S], BF16, tag="xkd_s")
        for j in range(4):  # split so ST/R matmuls unblock progressively
            nc.sync.dma_start(xT_s[:, j * 1024:(j + 1) * 1024],
                              xT[:, j * 1024:(j + 1) * 1024])
            nc.gpsimd.dma_start(xkd_s[:, j * 1024:(j + 1) * 1024],
                                xkd[:, j * 1024:(j + 1) * 1024])
        mT_s = wp_s[:, 0:128]     # [d2, d] = Wq^T Wk
        wv_s = wp_s[:, 128:256]   # [d, v] = Wv^T
        ones_s = wp_s[:, 256:384]

        QW_s = persist.tile([128, NQ], BF16, tag="QW_s")  # [d, q]

        # ---- QW projection: QW[:, jQB:(j+1)QB] = mT.T @ xqT block ----
        def qw_mm(j):
            pt = ps_st.tile([128, GROUP * QB], F32, tag="st", name=f"qw{j}")
            nc.tensor.matmul(pt[:, 0:QB], mT_s,
                             xqT_s[:, j * QB:(j + 1) * QB],
                             start=True, stop=True)
            return pt

        def qw_copy(j, pt):
            nc.vector.tensor_copy(QW_s[:, j * QB:(j + 1) * QB], pt[:, 0:QB])

        pt0 = qw_mm(0)
        nc.vector.tensor_copy(QW_s[:, 0:QB], pt0[:, 0:QB])  # critical path

        # ---- attention: flat software pipeline over (qblock, group) ----
        ALL = [(qb, g) for qb in range(NQB) for g in range(NGRP)]

        def emit_st(qb, g):
            q0 = qb * QB
            st = ps_st.tile([128, GROUP * QB], F32, tag="st")
            for i in range(GROUP):
                kc = g * GROUP + i
                nc.tensor.matmul(st[:, i * QB:(i + 1) * QB],
                                 xT_s[:, kc * KC:(kc + 1) * KC],
                                 QW_s[:, q0:q0 + QB],
                                 start=True, stop=True)
            return st

        st_tiles = {}
        for k in range(3):
            st_tiles[ALL[k]] = emit_st(*ALL[k])

        r_tiles = {}
        stacks = {qb: [] for qb in range(NQB)}  # binary-counter D trees
        nadd = {qb: 0 for qb in range(NQB)}

        def tree_add(qb, dst, a, b_, gp):
            # route a fraction of adds to GpSimd, but keep the late-qblock
            # adds (feeding the finish chain) on the faster DVE
            eng = nc.gpsimd if (nadd[qb] % 4 == 3 and gp < NGRP - 4) \
                else nc.vector
            eng.tensor_add(dst, a, b_)
            nadd[qb] += 1

        def push(qb, t, lvl, gp):
            st_ = stacks[qb]
            while st_ and st_[-1][0] == lvl:
                plvl, pt_ = st_.pop()
                nt = tpool.tile([128, QB], BF16, tag="tr")
                tree_add(qb, nt[:], pt_[:], t[:], gp)
                t, lvl = nt, plvl + 1
            st_.append((lvl, t))

        def collapse(qb, gp):
            st_ = stacks[qb]
            while len(st_) > 1:
                l0, t0 = st_.pop()
                l1, t_1 = st_.pop()
                nt = tpool.tile([128, QB], BF16, tag="tr")
                tree_add(qb, nt[:], t_1[:], t0[:], gp)
                st_.append((max(l0, l1) + 1, nt))
            return st_[0][1]

        # dribble the remaining QW blocks into early groups: mm at the
        # given idx, copy one idx later (QW j feeds ST group 16j,
        # emitted at idx 16j-3; earliest consumer is idx 13)
        qw_pend = {}
        QW_MM_AT = {0: 1, 4: 2, 8: 3}
        QW_CP_AT = {2: 1, 6: 2, 10: 3}

        for idx, (qb, g) in enumerate(ALL):
            st = st_tiles.pop((qb, g))
            p = ppool.tile([128, GROUP * QB], BF16, tag="p")
            nc.scalar.activation(p[:], st[:], EXP, scale=0.125)
            if qb not in r_tiles:
                r_tiles[qb] = ps_r.tile([128, QB], F32, tag="pv",
                                        name=f"r{qb}")
            rt = r_tiles[qb]
            for i in range(GROUP):
                kc = g * GROUP + i
                nc.tensor.matmul(rt[:], xkd_s[:, kc * KC:(kc + 1) * KC],
                                 p[:, i * QB:(i + 1) * QB],
                                 start=(kc == 0), stop=(kc == NKC - 1))
            if idx + 3 < len(ALL):
                st_tiles[ALL[idx + 3]] = emit_st(*ALL[idx + 3])
            if idx in QW_MM_AT:
                j = QW_MM_AT[idx]
                qw_pend[j] = qw_mm(j)
            if idx in QW_CP_AT:
                j = QW_CP_AT[idx]
                qw_copy(j, qw_pend.pop(j))

            if g < NGRP - 1:
                # leaf: sum the group's two chunks; feed the binary counter
                t1 = tpool.tile([128, QB], BF16, tag="tr")
                tree_add(qb, t1[:], p[:, 0:QB], p[:, QB:2 * QB], g)
                push(qb, t1, 1, g)
                if g == NGRP - 2:
                    # pre-collapse so the last group's chain is matmul-only
                    collapse(qb, g)
            else:
                # last group: fold its two chunks into the ones-matmul psum
                # accumulation -> D arrives broadcast across partitions.
                # dps shares a borrowed st-ring tile with the Wv^T R output.
                s_tile = stacks[qb].pop()[1]
                stacks[qb].clear()
                fin = ps_st.tile([128, GROUP * QB], F32, tag="st",
                                 name=f"fin{qb}")
                nc.tensor.matmul(fin[:, QB:2 * QB], ones_s, s_tile[:],
                                 start=True, stop=False)
                nc.tensor.matmul(fin[:, QB:2 * QB], ones_s, p[:, 0:QB],
                                 start=False, stop=False)
                nc.tensor.matmul(fin[:, QB:2 * QB], ones_s, p[:, QB:2 * QB],
                                 start=False, stop=True)
                rb = mpool.tile([128, QB], BF16, tag="rb")
                nc.vector.tensor_copy(rb[:], rt[:])  # frees the R psum bank
                nc.tensor.matmul(fin[:, 0:QB], wv_s, rb[:],
                                 start=True, stop=True)
                dinvb = mpool.tile([128, QB], F32, tag="dinvb")
                nc.vector.reciprocal_approx_fast(dinvb[:], fin[:, QB:2 * QB])
                outf = mpool.tile([128, QB], BF16, tag="outf")
                nc.vector.tensor_mul(outf[:], fin[:, 0:QB], dinvb[:])
                nc.sync.dma_start(out[:, qb * QB:(qb + 1) * QB], outf[:])
                del r_tiles[qb]

    _log("build_nc: bacc compile")
    nc.compile()
    _log("build_nc: done")
    return nc


def _host_prep(x, Wq, Wk, Wv):
    x = np.asarray(x, dtype=np.float32)
    Wq = np.asarray(Wq, dtype=np.float32)
    Wk = np.asarray(Wk, dtype=np.float32)
    Wv = np.asarray(Wv, dtype=np.float32)
    mT = Wq.T @ Wk                      # [d2, d]
    wpack = np.concatenate(
        [mT, Wv.T, np.ones((D, D), np.float32)], axis=1).astype(NPBF16)
    wpack = np.ascontiguousarray(wpack)
    in_maps = []
    for c in range(8):
        b, h = c // 2, c % 2
        xb = x[b]                       # [S, D]
        xkd = np.ascontiguousarray(
            xb.reshape(NKC, KC, D).transpose(1, 0, 2).reshape(KC, NKC * D)
        ).astype(NPBF16)                # [k, c*128+d]
        in_maps.append({
            "xT": np.ascontiguousarray(xb.T).astype(NPBF16),
            "xkd": xkd,
            "xqT": np.ascontiguousarray(
                xb[h * NQ:(h + 1) * NQ].T).astype(NPBF16),
            "wpack": wpack,
        })
    return in_maps


def run(x, Wq, Wk, Wv, trace=False, **kw):
    global _CACHED_NC
    if _CACHED_NC is None:
        _CACHED_NC = build_nc()
    in_maps = _host_prep(x, Wq, Wk, Wv)
    _log("run_bass_kernel_spmd (includes NEFF compile on first call)")
    res = run_bass_kernel_spmd(
        _CACHED_NC, in_maps, core_ids=list(range(8)), trace=trace, **kw)
    _log("run_bass_kernel_spmd returned")
    full = np.zeros((B, S, D), np.float32)
    for c in range(8):
        b, h = c // 2, c % 2
        full[b, h * NQ:(h + 1) * NQ] = np.asarray(
            res.results[c]["out"]).astype(np.float32).T
    return full, res


def kernel(x, Wq, Wk, Wv):
    full, _ = run(x, Wq, Wk, Wv, trace=False)
    return full


# revision 12
# speedup vs baseline: 1.2621x; 1.0487x over previous
"""Self-contained Trainium2 attention-block kernel (8 NeuronCores, SPMD).

Problem: x[4,4096,128], Wq/Wk[64,128], Wv[128,128] ->
  softmax((x Wq^T)(x Wk^T)^T / 8) (x Wv^T)   -> [4,4096,128] f32

Sharding: data-parallel over batch (4) x query-halves (2) = 8 cores.
Each core: q rows 2048, full K (4096) via algebra below. No collectives.

Algebraic reformulation (projections folded into attention):
  scores[k,q] = x_k^T (Wk^T Wq) x_q = xT_chunk.T @ QW,
    QW = M^T x_q with M^T = Wq^T Wk precomputed on host  -> no K proj.
  out[v,q] = Wv^T R / D with R[d,q] = sum_k x[k,d] p[k,q] accumulated
    like PV over k chunks                                -> no V proj.
Both ST and R matmuls contract over the full 128 dim with lhsT taken
straight from DMA'd x layouts (xT [d,k] and x_kd [k, c*128+d]).

Per-core pipeline (all matmuls bf16), ACT-exp-bound steady state:
  ST[k,q] psum ring-3 x [128,1024] (6 banks) + R psum 2x1 bank = 8.
  exp on ScalarE with fused 1/8 scale (no max-subtract: scores ~N(0,1)).
  Softmax denominator: bf16 binary-counter tree on DVE over groups
  0..14; the last group's two chunks fold into the ones-matmul psum
  accumulation (dps), which shares a borrowed ST-ring slot with the
  Wv^T R output. Inputs stream on four DMA queues, weights first.
"""

import sys

sys.path.insert(0, "/opt/trn_rl_repo")

from contextlib import ExitStack

import ml_dtypes
import numpy as np

import concourse.bass as bass  # noqa: F401
import concourse.bacc as bacc
import concourse.tile as tile
from concourse import mybir
from concourse.bass_utils import run_bass_kernel_spmd

BF16 = mybir.dt.bfloat16
F32 = mybir.dt.float32
NPBF16 = ml_dtypes.bfloat16

B, S, D, A = 4, 4096, 128, 64
NQ = S // 2          # q rows per core
QB = 512             # q block (psum bank free size)
KC = 128             # k chunk (matmul contraction tile)
NKC = S // KC        # 32 chunks
NQB = NQ // QB       # 4 q blocks
GROUP = 2            # k chunks per exp group ([128,1024] psum tile)
NGRP = NKC // GROUP  # 16 groups per block
EXP = mybir.ActivationFunctionType.Exp

_CACHED_NC = None


def _log(msg):
    import time as _t
    print(f"[kernel {_t.strftime('%H:%M:%S')}] {msg}", file=sys.stderr, flush=True)


def build_nc():
    _log("build_nc: tracing graph")
    nc = bacc.Bacc(
        "TRN2", target_bir_lowering=False, debug=False,
        enable_asserts=False, num_devices=8,
    )
    xT = nc.dram_tensor("xT", [D, S], BF16, kind="ExternalInput").ap()
    xkd = nc.dram_tensor("xkd", [128, S], BF16, kind="ExternalInput").ap()
    xqT = nc.dram_tensor("xqT", [D, NQ], BF16, kind="ExternalInput").ap()
    # mT = Wq^T Wk | wvT = Wv^T | ones, packed: one small DMA, first
    wpack = nc.dram_tensor("wpack", [D, 384], BF16, kind="ExternalInput").ap()
    # outT layout [v, q]; host transposes for free during gather
    out = nc.dram_tensor("out", [D, NQ], BF16, kind="ExternalOutput").ap()

    with tile.TileContext(nc) as tc, ExitStack() as ctx:
        persist = ctx.enter_context(tc.tile_pool(name="persist", bufs=1))
        # PSUM: st ring 3x(2 banks) + R 2x(1 bank) = 8 banks.
        # dps/WvR and QW-proj tiles borrow ring slots via matching tags.
        ps_st = ctx.enter_context(tc.tile_pool(name="ps_st", bufs=3, space="PSUM"))
        ps_r = ctx.enter_context(tc.tile_pool(name="ps_r", bufs=2, space="PSUM"))
        ppool = ctx.enter_context(tc.tile_pool(name="ppool", bufs=6))
        tpool = ctx.enter_context(tc.tile_pool(name="tpool", bufs=10))
        mpool = ctx.enter_context(tc.tile_pool(name="mpool", bufs=2))

        # ---- persistent SBUF + input DMAs ----
        # Queues share HBM bandwidth, so the critical-path transfers
        # (wpack -> qw0; xqT0, xT pieces -> ST stream) lead their queues
        # and the bulk xkd stream is gated behind wpack via a dummy read.
        wp_s = persist.tile([D, 384], BF16, tag="wp_s")
        nc.scalar.dma_start(wp_s[:], wpack[:])
        xqT_s = persist.tile([D, NQ], BF16, tag="xqT_s")
        nc.sync.dma_start(xqT_s[:, 0:QB], xqT[:, 0:QB])
        nc.scalar.dma_start(xqT_s[:, QB:], xqT[:, QB:])
        # prewarm the exp table before the gpsimd queue fills with DMA
        # enqueues, so the first real exp doesn't eat the ACT_TABLE_LOAD
        warm = persist.tile([1, 1], F32, tag="warm")
        nc.gpsimd.memset(warm[:], 1.0)
        warm2 = persist.tile([1, 1], F32, tag="warm2")
        nc.scalar.activation(warm2[:], warm[:], EXP)

        xT_s = persist.tile([D, S], BF16, tag="xT_s")
        xkd_s = persist.tile([128, S], BF16, tag="xkd_s")
        for j in range(4):  # split so ST matmuls unblock progressively
            nc.sync.dma_start(xT_s[:, j * 1024:(j + 1) * 1024],
                              xT[:, j * 1024:(j + 1) * 1024])
        gate = persist.tile([1, 1], BF16, tag="gate")
        nc.gpsimd.tensor_copy(gate[:], wp_s[0:1, 0:1])  # delay xkd pulls
        for j in range(4):
            nc.gpsimd.dma_start(xkd_s[:, j * 1024:(j + 1) * 1024],
                                xkd[:, j * 1024:(j + 1) * 1024])
        mT_s = wp_s[:, 0:128]     # [d2, d] = Wq^T Wk
        wv_s = wp_s[:, 128:256]   # [d, v] = Wv^T
        ones_s = wp_s[:, 256:384]

        QW_s = persist.tile([128, NQ], BF16, tag="QW_s")  # [d, q]

        # ---- QW projection: QW[:, jQB:(j+1)QB] = mT.T @ xqT block ----
        def qw_mm(j):
            pt = ps_st.tile([128, GROUP * QB], F32, tag="st", name=f"qw{j}")
            nc.tensor.matmul(pt[:, 0:QB], mT_s,
                             xqT_s[:, j * QB:(j + 1) * QB],
                             start=True, stop=True)
            return pt

        def qw_copy(j, pt):
            nc.vector.tensor_copy(QW_s[:, j * QB:(j + 1) * QB], pt[:, 0:QB])

        pt0 = qw_mm(0)
        nc.vector.tensor_copy(QW_s[:, 0:QB], pt0[:, 0:QB])  # critical path

        # ---- attention: flat software pipeline over (qblock, group) ----
        ALL = [(qb, g) for qb in range(NQB) for g in range(NGRP)]

        def emit_st(qb, g):
            q0 = qb * QB
            st = ps_st.tile([128, GROUP * QB], F32, tag="st")
            for i in range(GROUP):
                kc = g * GROUP + i
                nc.tensor.matmul(st[:, i * QB:(i + 1) * QB],
                                 xT_s[:, kc * KC:(kc + 1) * KC],
                                 QW_s[:, q0:q0 + QB],
                                 start=True, stop=True)
            return st

        st_tiles = {}
        for k in range(3):
            st_tiles[ALL[k]] = emit_st(*ALL[k])

        r_tiles = {}
        accs = {}  # qb -> running bf16 chunk-sum [128, QB]

        # dribble the remaining QW blocks into later groups (QW j feeds
        # ST group 16j, emitted at idx 16j-3), well behind the xqT DMA
        qw_pend = {}
        QW_MM_AT = {8: 1, 20: 2, 36: 3}
        QW_CP_AT = {10: 1, 22: 2, 38: 3}

        for idx, (qb, g) in enumerate(ALL):
            st = st_tiles.pop((qb, g))
            p = ppool.tile([128, GROUP * QB], BF16, tag="p")
            nc.scalar.activation(p[:], st[:], EXP, scale=0.125)
            if qb not in r_tiles:
                r_tiles[qb] = ps_r.tile([128, QB], F32, tag="pv",
                                        name=f"r{qb}")
            rt = r_tiles[qb]
            for i in range(GROUP):
                kc = g * GROUP + i
                nc.tensor.matmul(rt[:], xkd_s[:, kc * KC:(kc + 1) * KC],
                                 p[:, i * QB:(i + 1) * QB],
                                 start=(kc == 0), stop=(kc == NKC - 1))
            if idx + 3 < len(ALL):
                st_tiles[ALL[idx + 3]] = emit_st(*ALL[idx + 3])
            if idx in QW_MM_AT:
                j = QW_MM_AT[idx]
                qw_pend[j] = qw_mm(j)
            if idx in QW_CP_AT:
                j = QW_CP_AT[idx]
                qw_copy(j, qw_pend.pop(j))

            if g < NGRP - 1:
                # leaf: sum the group's two chunks (every 3rd on GpSimd),
                # then fold into the running bf16 accumulator on DVE
                t1 = tpool.tile([128, QB], BF16, tag="tr")
                leaf_eng = nc.gpsimd if g % 3 == 2 else nc.vector
                leaf_eng.tensor_add(t1[:], p[:, 0:QB], p[:, QB:2 * QB])
                if qb not in accs:
                    accs[qb] = t1
                else:
                    na = tpool.tile([128, QB], BF16, tag="tr")
                    nc.vector.tensor_add(na[:], accs[qb][:], t1[:])
                    accs[qb] = na
            else:
                # last group: fold its two chunks into the ones-matmul psum
                # accumulation -> D arrives broadcast across partitions.
                # dps shares a borrowed st-ring tile with the Wv^T R output.
                s_tile = accs.pop(qb)
                fin = ps_st.tile([128, GROUP * QB], F32, tag="st",
                                 name=f"fin{qb}")
                nc.tensor.matmul(fin[:, QB:2 * QB], ones_s, s_tile[:],
                                 start=True, stop=False)
                nc.tensor.matmul(fin[:, QB:2 * QB], ones_s, p[:, 0:QB],
                                 start=False, stop=False)
                nc.tensor.matmul(fin[:, QB:2 * QB], ones_s, p[:, QB:2 * QB],
                                 start=False, stop=True)
                rb = mpool.tile([128, QB], BF16, tag="rb")
                nc.vector.tensor_copy(rb[:], rt[:])  # frees the R psum bank
                nc.tensor.matmul(fin[:, 0:QB], wv_s, rb[:],
                                 start=True, stop=True)
                dinvb = mpool.tile([128, QB], F32, tag="dinvb")
                nc.vector.reciprocal_approx_fast(dinvb[:], fin[:, QB:2 * QB])
                outf = mpool.tile([128, QB], BF16, tag="outf")
                nc.vector.tensor_mul(outf[:], fin[:, 0:QB], dinvb[:])
                nc.sync.dma_start(out[:, qb * QB:(qb + 1) * QB], outf[:])
                del r_tiles[qb]

    _log("build_nc: bacc compile")
    nc.compile()
    _log("build_nc: done")
    return nc


def _host_prep(x, Wq, Wk, Wv):
    x = np.asarray(x, dtype=np.float32)
    Wq = np.asarray(Wq, dtype=np.float32)
    Wk = np.asarray(Wk, dtype=np.float32)
    Wv = np.asarray(Wv, dtype=np.float32)
    mT = Wq.T @ Wk                      # [d2, d]
    wpack = np.concatenate(
        [mT, Wv.T, np.ones((D, D), np.float32)], axis=1).astype(NPBF16)
    wpack = np.ascontiguousarray(wpack)
    in_maps = []
    for c in range(8):
        b, h = c // 2, c % 2
        xb = x[b]                       # [S, D]
        xkd = np.ascontiguousarray(
            xb.reshape(NKC, KC, D).transpose(1, 0, 2).reshape(KC, NKC * D)
        ).astype(NPBF16)                # [k, c*128+d]
        in_maps.append({
            "xT": np.ascontiguousarray(xb.T).astype(NPBF16),
            "xkd": xkd,
            "xqT": np.ascontiguousarray(
                xb[h * NQ:(h + 1) * NQ].T).astype(NPBF16),
            "wpack": wpack,
        })
    return in_maps


def run(x, Wq, Wk, Wv, trace=False, **kw):
    global _CACHED_NC
    if _CACHED_NC is None:
        _CACHED_NC = build_nc()
    in_maps = _host_prep(x, Wq, Wk, Wv)
    _log("run_bass_kernel_spmd (includes NEFF compile on first call)")
    res = run_bass_kernel_spmd(
        _CACHED_NC, in_maps, core_ids=list(range(8)), trace=trace, **kw)
    _log("run_bass_kernel_spmd returned")
    full = np.zeros((B, S, D), np.float32)
    for c in range(8):
        b, h = c // 2, c % 2
        full[b, h * NQ:(h + 1) * NQ] = np.asarray(
            res.results[c]["out"]).astype(np.float32).T
    return full, res


def kernel(x, Wq, Wk, Wv):
    full, _ = run(x, Wq, Wk, Wv, trace=False)
    return full
